# revision 7
# baseline (speedup 1.0000x reference)
"""Trainium2 Bass kernel for the CurriculumLoss nn.Module (count + Sinkhorn-OT + TV loss).

Key algebraic identity: the [4096,4096] Gibbs kernel over the 64x64 pooled grid
with squared-euclidean cost separates as a Kronecker product
    K = exp(-M/REG) = Ky (x) Kx,   Ky[i,j] = exp(-(i-j)^2/REG)  (64x64, Kx = Ky)
so K @ vec(V) == Ky @ V @ Kx for V the [64,64] image of v, and
    K*M = (Ky*My) (x) Kx  +  Ky (x) (Kx*Mx).
Each Sinkhorn half-step is then two 64x64x64 matmuls per sample instead of a
4096x4096 matvec (~32x fewer flops, and exact up to f32 rounding: the f32 Gibbs
kernel is 5-banded, all cross products match the dense entries to 1 ulp).

Sharding: data-parallel over the batch dim. 16 samples -> 8 cores x 2 samples.
Per-core layouts:
  pred  [128,1024]: partitions 0-63 sample0 / 64-127 sample1, partition p holds
        image rows 4*(p%64)..4*(p%64)+3 (so 4x4 avg-pool is a free-axis reduce).
  gt    [64,2048]:  partition u holds sample0 rows 4u..4u+3 then sample1 rows
        (pool lands directly in the [64(y), 2*64(x)] layout Sinkhorn wants).
Each core returns 16 partial scalars; the host combines them into the loss.
"""

import numpy as np

_N_CORES = 8
_ITERS = 50
_REG = 0.05

_CACHE = {}


def _consts():
    d = np.arange(64, dtype=np.float32)
    D = (d[:, None] - d[None, :]) ** 2  # exact small ints in f32
    Ky = np.exp(-(D / np.float32(_REG))).astype(np.float32)
    KM = (Ky * D).astype(np.float32)
    ident = np.eye(64, dtype=np.float32)
    sel = np.zeros((128, 2), np.float32)
    sel[:64, 0] = 1.0
    sel[64:, 1] = 1.0
    selt = np.ascontiguousarray(sel.T)
    return {"kmat": Ky, "km": KM, "ident": ident, "sel": sel, "selt": selt}


def _emit(tc, pred_d, gt_d, kmat_d, km_d, ident_d, sel_d, selt_d, out_d):
    from concourse import mybir

    nc = tc.nc
    f32 = mybir.dt.float32
    ALU = mybir.AluOpType
    ACTF = mybir.ActivationFunctionType
    AX = mybir.AxisListType

    with (
        tc.tile_pool(name="persist", bufs=1) as S,
        tc.tile_pool(name="work", bufs=3) as W,
        tc.tile_pool(name="ps", bufs=4, space="PSUM") as P,
        tc.tile_pool(name="ps_small", bufs=1, space="PSUM") as PS,
    ):
        # ---- loads ----
        pred = S.tile([128, 1024], f32, tag="pred")
        nc.sync.dma_start(out=pred[:], in_=pred_d)
        gt = S.tile([64, 2048], f32, tag="gt")
        nc.sync.dma_start(out=gt[:], in_=gt_d)
        kmat = S.tile([64, 64], f32, tag="kmat")
        nc.sync.dma_start(out=kmat[:], in_=kmat_d)
        km = S.tile([64, 64], f32, tag="km")
        nc.sync.dma_start(out=km[:], in_=km_d)
        ident = S.tile([64, 64], f32, tag="ident")
        nc.sync.dma_start(out=ident[:], in_=ident_d)
        sel = S.tile([128, 2], f32, tag="sel")
        nc.sync.dma_start(out=sel[:], in_=sel_d)
        selt = S.tile([2, 128], f32, tag="selt")
        nc.sync.dma_start(out=selt[:], in_=selt_d)

        # stats columns: 0 pc | 1 gc_s0 | 2 gc_s1 | 3 dx | 4 dy_within |
        #                5 dy_cross | 6 cost_s0 | 7 cost_s1
        stats = S.tile([128, 8], f32, tag="stats")
        nc.vector.memset(stats[:], 0.0)
        sums = S.tile([128, 4], f32, tag="sums")
        nc.vector.memset(sums[:], 1.0)

        # ---- 4x4 average pooling (sums; the /16 cancels in normalization) ----
        PA = S.tile([128, 64], f32, tag="PA")
        nc.vector.reduce_sum(
            PA[:],
            pred[:].rearrange("p (r g c) -> p g r c", r=4, g=64, c=4),
            axis=AX.XY,
        )
        PB = S.tile([64, 128], f32, tag="PB")
        nc.vector.reduce_sum(
            PB[:].rearrange("p (s g) -> p s g", s=2, g=64),
            gt[:].rearrange("p (s r g c) -> p s g r c", s=2, r=4, g=64, c=4),
            axis=AX.XY,
        )

        # ---- normalization factors ----
        nc.vector.reduce_sum(sums[:, 0:1], PA[:], axis=AX.X)
        nc.vector.reduce_sum(sums[0:64, 1:2], PB[:, 0:64], axis=AX.X)
        nc.vector.reduce_sum(sums[0:64, 2:3], PB[:, 64:128], axis=AX.X)
        ssp = PS.tile([2, 4], f32, tag="small")
        nc.tensor.matmul(ssp[:], sel[:], sums[:], start=True, stop=True)
        ssb = S.tile([2, 4], f32, tag="ssb")
        nc.vector.tensor_copy(ssb[:], ssp[:])
        rss = S.tile([2, 4], f32, tag="rss")
        nc.vector.reciprocal(rss[:], ssb[:])
        bcp = PS.tile([128, 4], f32, tag="small2")
        nc.tensor.matmul(bcp[:], selt[:], rss[:], start=True, stop=True)
        rbc = S.tile([128, 4], f32, tag="rbc")
        nc.vector.tensor_copy(rbc[:], bcp[:])

        # a (pred marginal) in [128(sample-stacked y), 64(x)]; b in [64(y), 2*64(x)]
        A = S.tile([128, 64], f32, tag="A")
        nc.scalar.activation(A[:], PA[:], ACTF.Relu, scale=rbc[:, 0:1])
        Bcat = S.tile([64, 128], f32, tag="Bcat")
        nc.scalar.activation(Bcat[:, 0:64], PB[:, 0:64], ACTF.Relu, scale=rbc[0:64, 1:2])
        nc.scalar.activation(
            Bcat[:, 64:128], PB[:, 64:128], ACTF.Relu, scale=rbc[0:64, 2:3]
        )

        # aT: per-sample transpose of A -> [64(x), 2*64(y)]
        # (PE needs lhsT/rhs at the same base partition: move sample1 down first)
        A1 = S.tile([64, 64], f32, tag="A1")
        nc.vector.tensor_copy(A1[:], A[64:128, :])
        psT = P.tile([64, 128], f32, tag="ps")
        nc.tensor.transpose(psT[:, 0:64], A[0:64, :], ident[:])
        nc.tensor.transpose(psT[:, 64:128], A1[:], ident[:])
        aT = S.tile([64, 128], f32, tag="aT")
        nc.vector.tensor_copy(aT[:], psT[:])

        # ---- counting-loss partials (ScalarE, fused accumulate) ----
        scrap = S.tile([128, 2048], f32, tag="scrap")
        nc.scalar.activation(
            scrap[:, 0:1024], pred[:], ACTF.Copy, accum_out=stats[:, 0:1]
        )
        nc.scalar.activation(
            scrap[0:64, 0:1024], gt[:, 0:1024], ACTF.Copy, accum_out=stats[0:64, 1:2]
        )
        nc.scalar.activation(
            scrap[0:64, 1024:2048],
            gt[:, 1024:2048],
            ACTF.Copy,
            accum_out=stats[0:64, 2:3],
        )

        # ---- total-variation partials ----
        predr = pred[:].rearrange("p (r c) -> p r c", r=4, c=256)
        dxd = S.tile([128, 1020], f32, tag="dxd")
        nc.vector.tensor_tensor(
            dxd[:].rearrange("p (r c) -> p r c", r=4, c=255),
            predr[:, :, 1:256],
            predr[:, :, 0:255],
            op=ALU.subtract,
        )
        nc.scalar.activation(
            scrap[:, 0:1020], dxd[:], ACTF.Abs, accum_out=stats[:, 3:4]
        )
        dyd = S.tile([128, 768], f32, tag="dyd")
        nc.vector.tensor_tensor(
            dyd[:], pred[:, 256:1024], pred[:, 0:768], op=ALU.subtract
        )
        nc.scalar.activation(scrap[:, 0:768], dyd[:], ACTF.Abs, accum_out=stats[:, 4:5])
        # cross-partition row pairs (row 4u+3 on partition p vs row 4u+4 on p+1).
        # DVE partition bases must be quadrant-aligned, so DMA the next-group
        # first rows shifted down by one partition; the last group of each
        # sample gets its own last row so that pair differences to exactly 0.
        shif = S.tile([128, 256], f32, tag="shif")
        nc.sync.dma_start(out=shif[0:63, :], in_=pred_d[1:64, 0:256])
        nc.sync.dma_start(out=shif[63:64, :], in_=pred_d[63:64, 768:1024])
        nc.sync.dma_start(out=shif[64:127, :], in_=pred_d[65:128, 0:256])
        nc.sync.dma_start(out=shif[127:128, :], in_=pred_d[127:128, 768:1024])
        dyc = S.tile([128, 256], f32, tag="dyc")
        nc.vector.tensor_tensor(dyc[:], shif[:], pred[:, 768:1024], op=ALU.subtract)
        nc.scalar.activation(
            scrap[:, 0:256], dyc[:], ACTF.Abs, accum_out=stats[:, 5:6]
        )

        # ---- Sinkhorn iterations ----
        # State V [64(y), 2*64(x)] ; Ut [64(x), 2*64(y)] (per-sample transposed).
        # u-half: T^T = Kx (V^T Ky) per sample, batched second matmul; u = a/T.
        # v-half: S = Ky (U Kx) per sample; v = b/S.
        V = S.tile([64, 128], f32, tag="V")
        nc.vector.memset(V[:], 1.0)
        Ut = S.tile([64, 128], f32, tag="Ut")

        for _ in range(_ITERS):
            qp = P.tile([64, 128], f32, tag="ps")
            nc.tensor.matmul(qp[:, 0:64], V[:, 0:64], kmat[:], start=True, stop=True)
            nc.tensor.matmul(qp[:, 64:128], V[:, 64:128], kmat[:], start=True, stop=True)
            qs = W.tile([64, 128], f32, tag="w")
            nc.scalar.activation(qs[:], qp[:], ACTF.Copy)
            tp = P.tile([64, 128], f32, tag="ps")
            nc.tensor.matmul(tp[:], kmat[:], qs[:], start=True, stop=True)
            rc = W.tile([64, 128], f32, tag="rc")
            nc.vector.reciprocal(rc[:], tp[:])
            nc.vector.tensor_mul(Ut[:], aT[:], rc[:])

            q2p = P.tile([64, 128], f32, tag="ps")
            nc.tensor.matmul(q2p[:, 0:64], Ut[:, 0:64], kmat[:], start=True, stop=True)
            nc.tensor.matmul(
                q2p[:, 64:128], Ut[:, 64:128], kmat[:], start=True, stop=True
            )
            q2s = W.tile([64, 128], f32, tag="w")
            nc.scalar.activation(q2s[:], q2p[:], ACTF.Copy)
            sp = P.tile([64, 128], f32, tag="ps")
            nc.tensor.matmul(sp[:], kmat[:], q2s[:], start=True, stop=True)
            rc2 = W.tile([64, 128], f32, tag="rc")
            nc.vector.reciprocal(rc2[:], sp[:])
            nc.vector.tensor_mul(V[:], Bcat[:], rc2[:])

        # ---- OT cost: sum(U o ((Ky*My) V Kx + Ky V (Kx*Mx))), in transposed layout
        qa = P.tile([64, 128], f32, tag="ps")
        nc.tensor.matmul(qa[:, 0:64], V[:, 0:64], km[:], start=True, stop=True)
        nc.tensor.matmul(qa[:, 64:128], V[:, 64:128], km[:], start=True, stop=True)
        qas = W.tile([64, 128], f32, tag="w")
        nc.scalar.activation(qas[:], qa[:], ACTF.Copy)
        qb = P.tile([64, 128], f32, tag="ps")
        nc.tensor.matmul(qb[:, 0:64], V[:, 0:64], kmat[:], start=True, stop=True)
        nc.tensor.matmul(qb[:, 64:128], V[:, 64:128], kmat[:], start=True, stop=True)
        qbs = W.tile([64, 128], f32, tag="w")
        nc.scalar.activation(qbs[:], qb[:], ACTF.Copy)
        cp = P.tile([64, 128], f32, tag="ps")
        nc.tensor.matmul(cp[:], kmat[:], qas[:], start=True, stop=False)
        nc.tensor.matmul(cp[:], km[:], qbs[:], start=False, stop=True)
        cs = W.tile([64, 128], f32, tag="w")
        nc.scalar.activation(cs[:], cp[:], ACTF.Copy)
        cscr = W.tile([64, 128], f32, tag="cscr")
        nc.vector.tensor_mul(cscr[:], Ut[:], cs[:])
        nc.vector.reduce_sum(stats[0:64, 6:7], cscr[:, 0:64], axis=AX.X)
        nc.vector.reduce_sum(stats[0:64, 7:8], cscr[:, 64:128], axis=AX.X)

        # ---- per-sample reduction of all partials and store ----
        op = PS.tile([2, 8], f32, tag="small3")
        nc.tensor.matmul(op[:], sel[:], stats[:], start=True, stop=True)
        ob = S.tile([2, 8], f32, tag="ob")
        nc.vector.tensor_copy(ob[:], op[:])
        nc.sync.dma_start(out=out_d, in_=ob[:])


def _build_program():
    import concourse.bacc as bacc
    import concourse.tile as tile
    from concourse import mybir

    f32 = mybir.dt.float32
    nc = bacc.Bacc(
        "TRN2",
        target_bir_lowering=False,
        debug=False,
        enable_asserts=False,
        num_devices=_N_CORES,
    )
    pred_d = nc.dram_tensor("pred", [128, 1024], f32, kind="ExternalInput").ap()
    gt_d = nc.dram_tensor("gt", [64, 2048], f32, kind="ExternalInput").ap()
    kmat_d = nc.dram_tensor("kmat", [64, 64], f32, kind="ExternalInput").ap()
    km_d = nc.dram_tensor("km", [64, 64], f32, kind="ExternalInput").ap()
    ident_d = nc.dram_tensor("ident", [64, 64], f32, kind="ExternalInput").ap()
    sel_d = nc.dram_tensor("sel", [128, 2], f32, kind="ExternalInput").ap()
    selt_d = nc.dram_tensor("selt", [2, 128], f32, kind="ExternalInput").ap()
    out_d = nc.dram_tensor("out", [2, 8], f32, kind="ExternalOutput").ap()

    with tile.TileContext(nc) as tc:
        _emit(tc, pred_d, gt_d, kmat_d, km_d, ident_d, sel_d, selt_d, out_d)
    nc.compile()
    return nc


def _get_nc():
    if "nc" not in _CACHE:
        _CACHE["nc"] = _build_program()
    return _CACHE["nc"]


def _make_in_maps(pred, gt):
    consts = _consts()
    maps = []
    for c in range(_N_CORES):
        p2 = np.ascontiguousarray(pred[2 * c : 2 * c + 2]).reshape(128, 1024)
        g = gt[2 * c : 2 * c + 2]
        g2 = np.ascontiguousarray(
            np.concatenate([g[0].reshape(64, 1024), g[1].reshape(64, 1024)], axis=1)
        )
        maps.append({"pred": p2, "gt": g2, **consts})
    return maps


def _run(in_maps, **kwargs):
    from concourse.bass_utils import run_bass_kernel_spmd

    return run_bass_kernel_spmd(
        _get_nc(), in_maps, core_ids=list(range(_N_CORES)), **kwargs
    )


def _finalize(results, t):
    pc = np.zeros(16, np.float32)
    gc = np.zeros(16, np.float32)
    dxs = np.zeros(16, np.float32)
    dys = np.zeros(16, np.float32)
    cost = np.zeros(16, np.float32)
    for c in range(_N_CORES):
        o = results[c]["out"]
        pc[2 * c] = o[0, 0]
        pc[2 * c + 1] = o[1, 0]
        gc[2 * c] = o[0, 1]
        gc[2 * c + 1] = o[0, 2]
        dxs[2 * c] = o[0, 3]
        dxs[2 * c + 1] = o[1, 3]
        dys[2 * c] = o[0, 4] + o[0, 5]
        dys[2 * c + 1] = o[1, 4] + o[1, 5]
        cost[2 * c] = o[0, 6]
        cost[2 * c + 1] = o[0, 7]
    l_count = np.abs(pc - gc).mean(dtype=np.float32)
    l_ot = cost.mean(dtype=np.float32)
    denom = np.float32(16 * 256 * 255)
    dx = np.float32(dxs.sum(dtype=np.float32) / denom)
    dy = np.float32(dys.sum(dtype=np.float32) / denom)
    l_tv = np.float32(dx + dy)
    w = np.float32(t)  # LAMBDA_OT = LAMBDA_TV = 1.0
    loss = np.float32(l_count + w * l_ot + w * l_tv)
    return np.array(loss, dtype=np.float32)


def kernel(pred, gt, epoch, max_epoch):
    pred = np.ascontiguousarray(np.asarray(pred, dtype=np.float32)).reshape(
        16, 256, 256
    )
    gt = np.ascontiguousarray(np.asarray(gt, dtype=np.float32)).reshape(16, 256, 256)
    t = float(int(np.asarray(epoch))) / float(max(1, int(np.asarray(max_epoch))))
    res = _run(_make_in_maps(pred, gt))
    return _finalize(res.results, t)
